# revision 15
# baseline (speedup 1.0000x reference)
"""SupCon loss (nn_ConLoss) on 8 Trainium2 NeuronCores.

Math: the reference builds logits = anchor @ contrast.T with anchor rows
being label-gathered prototypes, so logits has only N_CLASSES=100 distinct
rows.  Everything factors through P = protos @ contrast.T  [100, V*B]:

  per class c:  M[c]  = max_j P[c,j]
                E[c]  = sum_j exp((P[c,j]-M[c])/T)
                G[c]  = sum_{j: l_j==c} P[c,j]
  per column j: d[j]  = P[l_j, j]                (diagonal of the big logits)

  row i (label c=l_i):  S_i   = E[c]·exp(...) - exp(d_i/T - M[c]/T)
                        numer = G[c]/T - V·cnt[c]·M[c]/T - (d_i/T - M[c]/T)
                        mlpp  = numer/(V·cnt[c]-1) - log S_i
  loss = -mean(mlpp)

Sharding: the V*B = 8192 contrast columns are split 1024 per core (this is
simultaneously an anchor-row shard since row i pairs with column i).  Each
core computes P_shard = protos @ contrast_shard.T on the tensor engine plus
the per-class partial stats (max / exp-sum / masked sum) and the diagonal
gather (one-hot mask matmul).  The tiny [100]-sized partials are merged on
the host (the "all-reduce" of the scalar loss mean).
"""

import numpy as np

import bass_rust
import concourse.bass as bass
import concourse.mybir as mybir
import concourse.tile as tile
from concourse.vector_clock import ScopedClock
from concourse.bass_utils import run_bass_kernel_spmd

B, V, D = 4096, 2, 512
N_CLASSES = 100
TEMPERATURE = 0.07
N_CORES = 8
CPB = (V * B) // N_CORES          # contrast columns per core = 1024
KT = D // 128                     # K-tiles of 128 = 4

def _split_multi_waits(nc):
    """This walrus build rejects instructions carrying more than one sync
    wait.  Hoist extra waits onto same-engine NOPs inserted immediately
    before the instruction (waits execute in program order on the same
    sequencer, so semantics are unchanged)."""
    n = 0
    for f in nc.m.functions:
        for b in f.blocks:
            insts = b.instructions  # live list
            i = 0
            while i < len(insts):
                inst = insts[i]
                si = inst.sync_info
                waits = list(si.on_wait) if si and si.on_wait else []
                if len(waits) > 1:
                    inst.sync_info = bass_rust.SyncInfo(
                        on_wait=waits[-1:], on_update=list(si.on_update or [])
                    )
                    for w in waits[:-1]:
                        nop = mybir.InstNoOp(name=f"waitsplit-{n}", ins=[], outs=[])
                        n += 1
                        nop.engine = inst.engine
                        nop.sync_info = bass_rust.SyncInfo(on_wait=[w], on_update=[])
                        insts.insert(i, nop)
                        i += 1
                i += 1


_nc_cache = None


def _build_program():
    global _nc_cache
    if _nc_cache is not None:
        return _nc_cache

    from concourse.masks import make_identity

    f32 = mybir.dt.float32
    f32r = mybir.dt.float32r
    bf16 = mybir.dt.bfloat16
    u8 = mybir.dt.uint8
    nc = bass.Bass()
    # bf16 inputs: the matmul accumulates fp32 in PSUM; input rounding gives
    # ~1e-1 abs error on P (|P|~100), which the host combine averages down to
    # ~5e-5 relative on the scalar loss (measured) while halving the
    # DMA-bound input bytes.  ct layout: [p, n*KT*512 + a*512 + j] so each
    # 512-column half-pipeline loads with a single 512KB DMA (the HWDGE
    # fixed cost is ~625ns per dma_start -- fewer, bigger DMAs win).
    ct = nc.declare_dram_parameter("ct", [128, KT * CPB], bf16, isOutput=False)
    pt = nc.declare_dram_parameter("pt", [128, KT * N_CLASSES], bf16, isOutput=False)
    mask = nc.declare_dram_parameter("mask", [N_CLASSES, CPB], u8, isOutput=False)
    ones = nc.declare_dram_parameter("ones", [128, 1], f32r, isOutput=False)
    # single output: row 0 = diagonal, rows 1..8 = transposed per-half stats
    # [mx0, mx1, es0, es1, gs0, gs1, 0, 0] -- one DMA instead of two.
    out = nc.declare_dram_parameter("out", [9, CPB], f32, isOutput=True)

    inv_t = 1.0 / TEMPERATURE
    NH = CPB // 512  # halves
    HB = KT * 512    # packed columns per half

    with tile.TileContext(nc) as tc:
        with (
            tc.tile_pool(name="singles", bufs=1) as singles,
            tc.tile_pool(name="work", bufs=1) as work,
            tc.tile_pool(name="psum", bufs=1, space="PSUM") as psum,
        ):
            # --- input DMAs: pt, mask, then the two ct halves ---
            pt_t = singles.tile([128, KT * N_CLASSES], bf16)
            nc.sync.dma_start(out=pt_t, in_=pt[:, :])
            mask_t = work.tile([N_CLASSES, CPB], u8)
            nc.sync.dma_start(out=mask_t, in_=mask[:, :])
            ct_h = []
            for n in range(NH):
                t = work.tile([128, HB], bf16, name=f"cth{n}", tag=f"cth{n}")
                nc.sync.dma_start(out=t, in_=ct[:, n * HB : (n + 1) * HB])
                ct_h.append(t)

            # constants: ones ships from host (memset cannot write f32r),
            # identity is built on the otherwise-idle Pool engine
            ones_t = singles.tile([128, 1], f32r)
            nc.sync.dma_start(out=ones_t, in_=ones[:, :])
            ident_t = singles.tile([N_CLASSES, N_CLASSES], f32)
            make_identity(nc, ident_t)

            p_ps, d_ps = [], []
            for n in range(NH):
                p_ps.append(psum.tile([N_CLASSES, 512], f32, name=f"pps{n}", tag=f"pps{n}"))
                d_ps.append(psum.tile([1, 512], f32, name=f"dps{n}", tag=f"dps{n}"))
            # stats columns: 0+n mx_n, 2+n es_n, 4+n gs_n, 6:8 pad
            stats_t = work.tile([N_CLASSES, 8], f32)
            nc.vector.memset(stats_t, 0.0)
            negb = work.tile([N_CLASSES, 2], f32)
            exp_scratch = work.tile([N_CLASSES, CPB], f32)
            mp = work.tile([N_CLASSES, CPB], f32r)
            outb = work.tile([1, CPB], f32)

            # PE: all P matmuls first (so half 1 is never stuck behind
            # half 0's epilogue), then the diagonal one-hot matmuls.
            for n in range(NH):
                for a in range(KT):
                    nc.tensor.matmul(
                        p_ps[n],
                        lhsT=pt_t[:, a * N_CLASSES : (a + 1) * N_CLASSES],
                        rhs=ct_h[n][:, a * 512 : (a + 1) * 512],
                        start=(a == 0),
                        stop=(a == KT - 1),
                    )

            for n in range(NH):
                lo, hi = n * 512, (n + 1) * 512
                # DVE: masked P first (feeds the PE diagonal matmul), then max
                nc.vector.tensor_mul(mp[:, lo:hi], mask_t[:, lo:hi], p_ps[n])
                nc.vector.reduce_max(
                    stats_t[:, n : n + 1], p_ps[n], axis=mybir.AxisListType.X
                )
                # PE: diagonal gather (f32r: 1 cycle/row), ACT: PSUM bounce
                nc.tensor.matmul(
                    d_ps[n], lhsT=ones_t[:N_CLASSES, :], rhs=mp[:, lo:hi],
                    start=True, stop=True,
                )
                nc.scalar.copy(outb[0:1, lo:hi], d_ps[n])
                # ACT: exp((P - mx_n)/T) with fused row-sum
                nc.scalar.mul(negb[:, n : n + 1], stats_t[:, n : n + 1], -inv_t)
                nc.scalar.activation(
                    out=exp_scratch[:, lo:hi],
                    in_=p_ps[n],
                    func=mybir.ActivationFunctionType.Exp,
                    bias=negb[:, n : n + 1],
                    scale=inv_t,
                    accum_out=stats_t[:, 2 + n : 3 + n],
                )
                nc.vector.reduce_sum(
                    stats_t[:, 4 + n : 5 + n], mp[:, lo:hi], axis=mybir.AxisListType.X
                )

            # transpose stats [100, 8] -> [8, 100] so its DMA is 8 big
            # descriptors instead of 100 tiny ones; diag ships separately
            # (row 0) since compute engines cannot shift partitions.
            st_ps = psum.tile([8, N_CLASSES], f32)
            nc.tensor.transpose(st_ps, stats_t, ident_t)
            st_sb = work.tile([8, N_CLASSES], f32)
            nc.scalar.copy(st_sb, st_ps)
            nc.sync.dma_start(out=out[0:1, :], in_=outb[0:1, :])
            nc.scalar.dma_start(out=out[1:9, 0:N_CLASSES], in_=st_sb)

    _split_multi_waits(nc)
    _nc_cache = nc
    return nc


def _prep_inputs(features, labels, global_protos):
    """Build the per-core input maps (shard + pack layouts on host)."""
    import ml_dtypes

    bf16 = ml_dtypes.bfloat16
    feats = np.ascontiguousarray(features, dtype=np.float32)
    protos = np.ascontiguousarray(global_protos, dtype=np.float32)
    labels = np.asarray(labels).astype(np.int64)

    # protosT [D, N] packed to [128, KT*N]: pt[p, a*N+c] = protos[c, a*128+p]
    pt = np.ascontiguousarray(
        protos.T.reshape(KT, 128, N_CLASSES).transpose(1, 0, 2).reshape(128, -1)
    ).astype(bf16)

    in_maps = []
    bpc = B // (N_CORES // V)  # batch rows per core slab = 1024
    for k in range(N_CORES):
        b0 = bpc * (k % (N_CORES // V))
        v = k // (N_CORES // V)
        slab = feats[b0 : b0 + bpc, v, :]  # [1024, 512]
        lab = labels[b0 : b0 + bpc]
        # contrastT packed [p, n*KT*512 + a*512 + j] (n-major halves)
        ct = np.ascontiguousarray(
            slab.T.reshape(KT, 128, CPB // 512, 512)
            .transpose(1, 2, 0, 3)
            .reshape(128, -1)
        ).astype(bf16)
        msk = (lab[None, :] == np.arange(N_CLASSES)[:, None]).astype(np.uint8)
        in_maps.append(
            {
                "ct": ct,
                "pt": pt,
                "mask": np.ascontiguousarray(msk),
                "ones": np.ones((128, 1), dtype=np.float32),
            }
        )
    return in_maps, labels


def _combine(results, labels):
    """Merge per-core/per-half partials into the scalar loss (float64)."""
    T = TEMPERATURE
    # out rows: 0 diag, 1+2 mx halves, 3+4 es halves, 5+6 gs halves
    mx_a = np.concatenate(
        [r["out"][1:3, :N_CLASSES] for r in results]
    ).astype(np.float64)                                         # [16, 100]
    es_a = np.concatenate(
        [r["out"][3:5, :N_CLASSES] for r in results]
    ).astype(np.float64)
    gs_a = np.concatenate(
        [r["out"][5:7, :N_CLASSES] for r in results]
    ).astype(np.float64)
    d = np.concatenate([r["out"][0] for r in results]).astype(np.float64)

    m = mx_a.max(axis=0)                                         # [100]
    E = (es_a * np.exp((mx_a - m[None, :]) / T)).sum(axis=0)     # [100]
    G = gs_a.sum(axis=0)                                         # [100]
    cnt = np.bincount(labels, minlength=N_CLASSES).astype(np.float64)

    lfull = np.tile(labels, V)                                   # [8192]
    mT = m[lfull] / T
    dT = d / T
    S = E[lfull] - np.exp(np.minimum(dT - mT, 0.0))
    S = np.maximum(S, 1e-300)
    npos = V * cnt[lfull] - 1.0
    numer = G[lfull] / T - V * cnt[lfull] * mT - (dT - mT)
    mlpp = numer / npos - np.log(S)
    return np.float32(-np.mean(mlpp))


def run(features, labels, global_protos, trace=False):
    nc = _build_program()
    in_maps, labels64 = _prep_inputs(features, labels, global_protos)
    res = run_bass_kernel_spmd(nc, in_maps, list(range(N_CORES)), trace=trace)
    loss = _combine(res.results, labels64)
    return loss, res


def kernel(features, labels, global_protos):
    loss, _ = run(features, labels, global_protos)
    return np.array(loss, dtype=np.float32)
